# revision 12
# baseline (speedup 1.0000x reference)
"""CostVolume (81-displacement, L2-normalized, leaky-relu) Trainium2 kernel.

Full inputs (B=4, C=128, H=128, W=256) are sharded across 8 NeuronCores:
batch x H-half data parallel (core k -> b=k//2, h-half=k%2). The +-4 halo in H
is handled host-side by slicing a zero-padded feat2; no collectives.

v2: bf16 datapath. Inputs are cast to bf16 on the host (halves input DMA and
runs every matmul at 1 cycle/row instead of fp32's 4). Per core:
  inv2 = abs_rsqrt(colsum(f2^2))    (one ACT pass straight out of PSUM)
  f2n  = f2 * inv2                  (DVE bf16 2x)
  n1col[w,2h+g] = rsqrt(C^2*|f1|^2) (256 small matmuls + one ACT)
  pband[p=(j,i), dy*40+u] = sum_c f1[c,h,A+p] * f2n[c,h+dy,A+32j+u]
      via 4 col-tiled bf16 matmuls (tile_position=(0,32j), N=360)
  band[p, hh, dy, u] = Prelu(pband * n1col) in NATIVE (dy,u) order (no
      transposed ACT writes), rows split between ScalarE and VectorE
and dumps bf16 full-band tiles [128, 32h, 360] (one descriptor/partition);
the host extracts the 9 dx diagonals (u = i + dx + 4) to build (B,81,H,W).
"""
import numpy as np
import ml_dtypes

import concourse.bass as bass
import concourse.bacc as bacc
import concourse.tile as tile
from concourse import mybir
from concourse.bass_utils import run_bass_kernel_spmd

F32 = mybir.dt.float32
BF16 = mybir.dt.bfloat16
NPBF16 = ml_dtypes.bfloat16
B, C, H, W = 4, 128, 128, 256
D = 4
HS = 64          # h rows per core
HP, WP = HS + 2 * D, W + 2 * D   # padded f2 shard dims: 72, 264

ACT_ROWS = 4     # of every 8 h-rows, this many evacuate via ScalarE (rest DVE)

_CACHE = {}


def _build():
    nc = bacc.Bacc("TRN2", target_bir_lowering=False, debug=False)

    f1_d = nc.dram_tensor("f1", [C, HS, W], BF16, kind="ExternalInput")
    f2_d = nc.dram_tensor("f2", [C, HP, WP], BF16, kind="ExternalInput")
    bo_d = nc.dram_tensor("bandout", [2, 2, 128, 32, 360], BF16,
                          kind="ExternalOutput")
    n1_d = nc.dram_tensor("n1out", [128, 128], F32, kind="ExternalOutput")

    with tile.TileContext(nc) as tc:
        with (
            tc.tile_pool(name="big", bufs=1) as big,
            tc.tile_pool(name="f2c", bufs=2) as f2cp,
            tc.tile_pool(name="sq2", bufs=2) as sq2p,
            tc.tile_pool(name="inv", bufs=2) as invp,
            tc.tile_pool(name="sq1", bufs=2) as sq1p,
            tc.tile_pool(name="tmp", bufs=2) as tmpp,
            tc.tile_pool(name="band", bufs=2) as bandp,
            tc.tile_pool(name="small", bufs=1) as smallp,
            tc.tile_pool(name="psb", bufs=2, space="PSUM") as psb,
            tc.tile_pool(name="psn", bufs=1, space="PSUM") as psn,
            tc.tile_pool(name="psn1", bufs=1, space="PSUM") as psn1,
        ):
            ones128 = smallp.tile([128, 128], BF16)
            nc.vector.memset(ones128[:], 1.0)
            onescol = smallp.tile([128, 1], BF16)
            nc.vector.memset(onescol[:], 1.0)
            eps128 = smallp.tile([128, 1], F32)
            nc.vector.memset(eps128[:], 1e-20)

            # ---- f1 in (2 DMAs) ----
            f1 = big.tile([C, HS, W], BF16)
            nc.sync.dma_start(out=f1[:, 0:32, :], in_=f1_d[:, 0:32, :])
            nc.sync.dma_start(out=f1[:, 32:64, :], in_=f1_d[:, 32:64, :])

            # ---- f2 in + normalize into f2n ----
            f2n = big.tile([C, HP, WP], BF16)
            NCH = 8                      # f2 rows per dma chunk
            SUB = 352                    # norm sub-chunk (6 per chunk)
            for ci in range(HP // NCH):
                f2c = f2cp.tile([C, NCH, WP], BF16)
                nc.sync.dma_start(out=f2c[:], in_=f2_d[:, NCH * ci : NCH * ci + NCH, :])
                f2cf = f2c[:].rearrange("c h w -> c (h w)")
                base = ci * NCH * WP
                for k in range(6):
                    sl = slice(k * SUB, (k + 1) * SUB)
                    sq2 = sq2p.tile([C, SUB], BF16)
                    nc.gpsimd.tensor_mul(out=sq2[:], in0=f2cf[:, sl], in1=f2cf[:, sl])
                    pbc = psn.tile([128, SUB], F32)
                    nc.tensor.matmul(pbc[:], ones128[:], sq2[:], start=True, stop=True)
                    inv = invp.tile([128, SUB], BF16)
                    nc.scalar.activation(
                        out=inv[:], in_=pbc[:],
                        func=mybir.ActivationFunctionType.Abs_reciprocal_sqrt,
                        bias=eps128[:])
                    f2nf = f2n[:].rearrange("c h w -> c (h w)")
                    nc.vector.tensor_mul(out=f2nf[:, base + k * SUB : base + (k + 1) * SUB],
                                         in0=f2cf[:, sl], in1=inv[:])

            # ---- n1col[w, 2h+g] = 1/(C*||f1(h, 128g+w)||) ----
            pn1 = psn1.tile([128, 128], F32)
            for hc in range(16):     # sq1 chunks of 4 h rows
                sq1 = sq1p.tile([C, 4, W], BF16)
                nc.vector.tensor_mul(out=sq1[:], in0=f1[:, 4 * hc : 4 * hc + 4, :],
                                     in1=f1[:, 4 * hc : 4 * hc + 4, :])
                for hh in range(4):
                    h = 4 * hc + hh
                    for g in range(2):
                        nc.tensor.matmul(
                            pn1[:, 2 * h + g : 2 * h + g + 1],
                            sq1[:, hh, 128 * g : 128 * g + 128],
                            onescol[:], start=True, stop=True)
            n1col = smallp.tile([128, 128], F32)
            nc.scalar.activation(out=n1col[:], in_=pn1[:],
                                 func=mybir.ActivationFunctionType.Abs_reciprocal_sqrt,
                                 scale=float(C * C))
            nc.sync.dma_start(out=n1_d[:], in_=n1col[:])

            # ---- gram + evac (native (dy,u) order) + big band dumps ----
            for g in range(2):
                A = 128 * g
                for half in range(2):
                    band = bandp.tile([128, 32, 360], BF16)
                    # groups of 3 h-rows share one PSUM mega-tile (bank-
                    # aligned via padded_shape) and one evac instruction;
                    # n1 scale is applied host-side (n1*leaky(x)==leaky(n1*x))
                    for t, hh0 in enumerate(range(0, 32, 3)):
                        nh = min(3, 32 - hh0)
                        pband = psb.tile([128, 3, 360], F32,
                                         padded_shape=[128, 3, 512])
                        for m in range(nh):
                            h = 32 * half + hh0 + m
                            for j in range(4):
                                w0 = A + 32 * j
                                nc.tensor.matmul(
                                    pband[32 * j : 32 * j + 32, m, :],
                                    f1[:, h, w0 : w0 + 32],
                                    f2n[:, h : h + 9, w0 : w0 + 40],
                                    start=True, stop=True,
                                    tile_position=(0, 32 * j))
                        if t % 4 < 3:   # ScalarE evacs 3 of 4 groups
                            nc.scalar.activation(
                                out=band[:, hh0 : hh0 + nh, :],
                                in_=pband[:, 0:nh, :],
                                func=mybir.ActivationFunctionType.Prelu,
                                alpha=0.1)
                        else:           # VectorE: copy out of PSUM, then leaky
                            tmp = tmpp.tile([128, 3, 360], BF16)
                            nc.vector.tensor_scalar_mul(
                                out=tmp[:, 0:nh, :], in0=pband[:, 0:nh, :],
                                scalar1=1.0)
                            nc.vector.scalar_tensor_tensor(
                                out=band[:, hh0 : hh0 + nh, :],
                                in0=tmp[:, 0:nh, :], scalar=0.1,
                                in1=tmp[:, 0:nh, :],
                                op0=mybir.AluOpType.mult,
                                op1=mybir.AluOpType.max)
                    nc.sync.dma_start(out=bo_d[g, half], in_=band[:])

    nc.compile()
    return nc


def _get_nc():
    if "nc" not in _CACHE:
        _CACHE["nc"] = _build()
    return _CACHE["nc"]


def _shard_inputs(feat1, feat2_warped):
    feat1 = np.asarray(feat1, dtype=np.float32)
    feat2 = np.asarray(feat2_warped, dtype=np.float32)
    f2pad = np.pad(feat2, ((0, 0), (0, 0), (D, D), (D, D)))
    f1b = feat1.astype(NPBF16)
    f2b = f2pad.astype(NPBF16)
    in_maps = []
    for k in range(8):
        b, s = k // 2, k % 2
        in_maps.append({
            "f1": np.ascontiguousarray(f1b[b, :, HS * s : HS * s + HS, :]),
            "f2": np.ascontiguousarray(f2b[b, :, HS * s : HS * s + HP, :]),
        })
    return in_maps


# u-window per weight column i: u = i + (dx+4), dx+4 in [0, 9)
_UIDX = (np.arange(32)[:, None] + np.arange(9)[None, :])  # [32 i, 9 dxi]


def _gather(results):
    out = np.empty((B, 81, H, W), dtype=np.float32)
    for k in range(8):
        b, s = k // 2, k % 2
        band = np.asarray(results[k]["bandout"], dtype=np.float32)
        # n1col[p, 2h+g] -> n1hw[h, 128g+p]
        n1 = np.asarray(results[k]["n1out"], dtype=np.float32)
        n1hw = n1.reshape(128, 64, 2).transpose(1, 2, 0).reshape(64, 256)
        # band: [2 g, 2 half, 128 p, 32 hh, 360 (dy*40+u)]
        band = band.reshape(2, 2, 4, 32, 32, 9, 40)  # g half j i hh dy u
        sel = np.take_along_axis(
            band, _UIDX[None, None, None, :, None, None, :], axis=6
        )  # [g, half, j, i, hh, dy, dxi]
        # -> [g, half, dy, dxi, hh, j, i]
        t = sel.transpose(0, 1, 5, 6, 4, 2, 3)
        for g in range(2):
            for half in range(2):
                out[b, :, HS * s + 32 * half : HS * s + 32 * half + 32,
                    128 * g : 128 * g + 128] = t[g, half].reshape(81, 32, 128)
        out[b, :, HS * s : HS * s + HS, :] *= n1hw[None, :, :]
    return out


def run(feat1, feat2_warped, trace=False):
    nc = _get_nc()
    in_maps = _shard_inputs(feat1, feat2_warped)
    res = run_bass_kernel_spmd(nc, in_maps, list(range(8)), trace=trace)
    return _gather(res.results), res


def kernel(feat1, feat2_warped):
    out, _ = run(feat1, feat2_warped)
    return out


# revision 15
# speedup vs baseline: 1.0201x; 1.0201x over previous
"""CostVolume (81-displacement, L2-normalized, leaky-relu) Trainium2 kernel.

Full inputs (B=4, C=128, H=128, W=256) are sharded across 8 NeuronCores:
batch x H-half data parallel (core k -> b=k//2, h-half=k%2). The +-4 halo in H
is handled host-side by slicing a zero-padded feat2; no collectives.

v2: bf16 datapath. Inputs are cast to bf16 on the host (halves input DMA and
runs every matmul at 1 cycle/row instead of fp32's 4). Per core:
  inv2 = abs_rsqrt(colsum(f2^2))    (one ACT pass straight out of PSUM)
  f2n  = f2 * inv2                  (DVE bf16 2x)
  n1col[w,2h+g] = rsqrt(C^2*|f1|^2) (256 small matmuls + one ACT)
  pband[p=(j,i), dy*40+u] = sum_c f1[c,h,A+p] * f2n[c,h+dy,A+32j+u]
      via 4 col-tiled bf16 matmuls (tile_position=(0,32j), N=360)
  band[p, hh, dy, u] = Prelu(pband * n1col) in NATIVE (dy,u) order (no
      transposed ACT writes), rows split between ScalarE and VectorE
and dumps bf16 full-band tiles [128, 32h, 360] (one descriptor/partition);
the host extracts the 9 dx diagonals (u = i + dx + 4) to build (B,81,H,W).
"""
import numpy as np
import ml_dtypes

import concourse.bass as bass
import concourse.bacc as bacc
import concourse.tile as tile
from concourse import mybir
from concourse.bass_utils import run_bass_kernel_spmd

F32 = mybir.dt.float32
BF16 = mybir.dt.bfloat16
NPBF16 = ml_dtypes.bfloat16
B, C, H, W = 4, 128, 128, 256
D = 4
HS = 64          # h rows per core
HP, WP = HS + 2 * D, W + 2 * D   # padded f2 shard dims: 72, 264

ACT_ROWS = 4     # of every 8 h-rows, this many evacuate via ScalarE (rest DVE)

_CACHE = {}


def _build():
    nc = bacc.Bacc("TRN2", target_bir_lowering=False, debug=False)

    f1_d = nc.dram_tensor("f1", [C, HS, W], BF16, kind="ExternalInput")
    f2_d = nc.dram_tensor("f2", [C, HP, WP], BF16, kind="ExternalInput")
    bo_d = nc.dram_tensor("bandout", [2, 2, 128, 32, 360], BF16,
                          kind="ExternalOutput")
    n1_d = nc.dram_tensor("n1out", [128, 128], F32, kind="ExternalOutput")

    with tile.TileContext(nc) as tc:
        with (
            tc.tile_pool(name="big", bufs=1) as big,
            tc.tile_pool(name="f2c", bufs=2) as f2cp,
            tc.tile_pool(name="sq2", bufs=2) as sq2p,
            tc.tile_pool(name="inv", bufs=2) as invp,
            tc.tile_pool(name="sq1", bufs=2) as sq1p,
            tc.tile_pool(name="tmp", bufs=2) as tmpp,
            tc.tile_pool(name="band", bufs=2) as bandp,
            tc.tile_pool(name="small", bufs=1) as smallp,
            tc.tile_pool(name="psb", bufs=2, space="PSUM") as psb,
            tc.tile_pool(name="psn", bufs=2, space="PSUM") as psn,
            tc.tile_pool(name="psn1", bufs=1, space="PSUM") as psn1,
        ):
            ones128 = smallp.tile([128, 128], BF16)
            nc.vector.memset(ones128[:], 1.0)
            onescol = smallp.tile([128, 1], BF16)
            nc.vector.memset(onescol[:], 1.0)
            eps128 = smallp.tile([128, 1], F32)
            nc.vector.memset(eps128[:], 1e-20)

            # ---- f1 in (2 DMAs) ----
            f1 = big.tile([C, HS, W], BF16)
            nc.sync.dma_start(out=f1[:, 0:32, :], in_=f1_d[:, 0:32, :])
            nc.sync.dma_start(out=f1[:, 32:64, :], in_=f1_d[:, 32:64, :])

            # ---- f2 in + normalize into f2n ----
            f2n = big.tile([C, HP, WP], BF16)
            NCH = 8                      # f2 rows per dma chunk
            SUB = 352                    # norm sub-chunk (6 per chunk)
            for ci in range(HP // NCH):
                f2c = f2cp.tile([C, NCH, WP], BF16)
                nc.sync.dma_start(out=f2c[:], in_=f2_d[:, NCH * ci : NCH * ci + NCH, :])
                f2cf = f2c[:].rearrange("c h w -> c (h w)")
                base = ci * NCH * WP
                for k in range(6):
                    sl = slice(k * SUB, (k + 1) * SUB)
                    sq2 = sq2p.tile([C, SUB], BF16)
                    nc.gpsimd.tensor_mul(out=sq2[:], in0=f2cf[:, sl], in1=f2cf[:, sl])
                    pbc = psn.tile([128, SUB], F32)
                    nc.tensor.matmul(pbc[:], ones128[:], sq2[:], start=True, stop=True)
                    inv = invp.tile([128, SUB], BF16)
                    nc.scalar.activation(
                        out=inv[:], in_=pbc[:],
                        func=mybir.ActivationFunctionType.Abs_reciprocal_sqrt,
                        bias=eps128[:])
                    f2nf = f2n[:].rearrange("c h w -> c (h w)")
                    nc.vector.tensor_mul(out=f2nf[:, base + k * SUB : base + (k + 1) * SUB],
                                         in0=f2cf[:, sl], in1=inv[:])

            # ---- n1col[w, 2h+g] = 1/(C*||f1(h, 128g+w)||) ----
            pn1 = psn1.tile([128, 128], F32)
            for hc in range(16):     # sq1 chunks of 4 h rows
                sq1 = sq1p.tile([C, 4, W], BF16)
                nc.vector.tensor_mul(out=sq1[:], in0=f1[:, 4 * hc : 4 * hc + 4, :],
                                     in1=f1[:, 4 * hc : 4 * hc + 4, :])
                for hh in range(4):
                    h = 4 * hc + hh
                    for g in range(2):
                        nc.tensor.matmul(
                            pn1[:, 2 * h + g : 2 * h + g + 1],
                            sq1[:, hh, 128 * g : 128 * g + 128],
                            onescol[:], start=True, stop=True)
            n1col = smallp.tile([128, 128], F32)
            nc.scalar.activation(out=n1col[:], in_=pn1[:],
                                 func=mybir.ActivationFunctionType.Abs_reciprocal_sqrt,
                                 scale=float(C * C))
            nc.sync.dma_start(out=n1_d[:], in_=n1col[:])

            # ---- gram + evac (native (dy,u) order) + big band dumps ----
            for g in range(2):
                A = 128 * g
                for half in range(2):
                    band = bandp.tile([128, 32, 360], BF16)
                    # groups of 3 h-rows share one PSUM mega-tile (bank-
                    # aligned via padded_shape) and one evac instruction;
                    # n1 scale is applied host-side (n1*leaky(x)==leaky(n1*x))
                    for t, hh0 in enumerate(range(0, 32, 2)):
                        nh = min(2, 32 - hh0)
                        pband = psb.tile([128, 2, 360], F32,
                                         padded_shape=[128, 2, 512])
                        for m in range(nh):
                            h = 32 * half + hh0 + m
                            for j in range(4):
                                w0 = A + 32 * j
                                nc.tensor.matmul(
                                    pband[32 * j : 32 * j + 32, m, :],
                                    f1[:, h, w0 : w0 + 32],
                                    f2n[:, h : h + 9, w0 : w0 + 40],
                                    start=True, stop=True,
                                    tile_position=(0, 32 * j))
                        if t % 4 < 3:   # ScalarE evacs 3 of 4 groups
                            nc.scalar.activation(
                                out=band[:, hh0 : hh0 + nh, :],
                                in_=pband[:, 0:nh, :],
                                func=mybir.ActivationFunctionType.Prelu,
                                alpha=0.1)
                        else:           # VectorE: copy out of PSUM, then leaky
                            tmp = tmpp.tile([128, 2, 360], BF16)
                            nc.vector.tensor_scalar_mul(
                                out=tmp[:, 0:nh, :], in0=pband[:, 0:nh, :],
                                scalar1=1.0)
                            nc.vector.scalar_tensor_tensor(
                                out=band[:, hh0 : hh0 + nh, :],
                                in0=tmp[:, 0:nh, :], scalar=0.1,
                                in1=tmp[:, 0:nh, :],
                                op0=mybir.AluOpType.mult,
                                op1=mybir.AluOpType.max)
                    nc.sync.dma_start(out=bo_d[g, half], in_=band[:])

    nc.compile()
    return nc


def _get_nc():
    if "nc" not in _CACHE:
        _CACHE["nc"] = _build()
    return _CACHE["nc"]


def _shard_inputs(feat1, feat2_warped):
    feat1 = np.asarray(feat1, dtype=np.float32)
    feat2 = np.asarray(feat2_warped, dtype=np.float32)
    f2pad = np.pad(feat2, ((0, 0), (0, 0), (D, D), (D, D)))
    f1b = feat1.astype(NPBF16)
    f2b = f2pad.astype(NPBF16)
    in_maps = []
    for k in range(8):
        b, s = k // 2, k % 2
        in_maps.append({
            "f1": np.ascontiguousarray(f1b[b, :, HS * s : HS * s + HS, :]),
            "f2": np.ascontiguousarray(f2b[b, :, HS * s : HS * s + HP, :]),
        })
    return in_maps


# u-window per weight column i: u = i + (dx+4), dx+4 in [0, 9)
_UIDX = (np.arange(32)[:, None] + np.arange(9)[None, :])  # [32 i, 9 dxi]


def _gather(results):
    out = np.empty((B, 81, H, W), dtype=np.float32)
    for k in range(8):
        b, s = k // 2, k % 2
        band = np.asarray(results[k]["bandout"], dtype=np.float32)
        # n1col[p, 2h+g] -> n1hw[h, 128g+p]
        n1 = np.asarray(results[k]["n1out"], dtype=np.float32)
        n1hw = n1.reshape(128, 64, 2).transpose(1, 2, 0).reshape(64, 256)
        # band: [2 g, 2 half, 128 p, 32 hh, 360 (dy*40+u)]
        band = band.reshape(2, 2, 4, 32, 32, 9, 40)  # g half j i hh dy u
        sel = np.take_along_axis(
            band, _UIDX[None, None, None, :, None, None, :], axis=6
        )  # [g, half, j, i, hh, dy, dxi]
        # -> [g, half, dy, dxi, hh, j, i]
        t = sel.transpose(0, 1, 5, 6, 4, 2, 3)
        for g in range(2):
            for half in range(2):
                out[b, :, HS * s + 32 * half : HS * s + 32 * half + 32,
                    128 * g : 128 * g + 128] = t[g, half].reshape(81, 32, 128)
        out[b, :, HS * s : HS * s + HS, :] *= n1hw[None, :, :]
    return out


def run(feat1, feat2_warped, trace=False):
    nc = _get_nc()
    in_maps = _shard_inputs(feat1, feat2_warped)
    res = run_bass_kernel_spmd(nc, in_maps, list(range(8)), trace=trace)
    return _gather(res.results), res


def kernel(feat1, feat2_warped):
    out, _ = run(feat1, feat2_warped)
    return out
